# revision 3
# baseline (speedup 1.0000x reference)
"""Trainium2 Bass kernel for masked multi-head attention (B=4, N=1024, D=1024, H=16).

Sharding: 8 cores = 4 batches x 2 head-groups (tensor parallel over heads).
Each core computes the QKV projection for its 8 heads, full attention, and a
partial output projection; the host sums the two partials per batch (+bout).

Device-side layout tricks:
- x is pre-transposed on the host; QKV projection computes qkv^T = W^T @ x^T
  so q/k land feature-major (per-head [64, N] tiles) and v token-major.
- The pairwise mask is folded into the scores matmul as a 65th contraction
  row (q_aug row = 1, k_aug row = 0/-30000 additive mask).
- Rows with mask[i]=False must produce a uniform 1/N attention row: the exp
  gets a per-partition bias of +86 on those rows, which overflows the row sum
  to inf -> reciprocal 0, and a fused DVE tensor_scalar blends in 1/N.
- All matmuls run in float32r (full-rate fp32 mode, ~1e-4 accuracy).

Returns (out, attn): out [4,1024,1024] f32, attn [4,16,1024,1024] f32.
"""

from contextlib import ExitStack

import numpy as np

import concourse.bass as bass
import concourse.tile as tile
from concourse import bacc, mybir
from concourse.bass_utils import run_bass_kernel_spmd

B, N, D, H = 4, 1024, 1024, 16
DH = D // H  # 64
HL = H // 2  # heads per core = 8
SCALE = D ** -0.5
NEG = -30000.0  # additive column mask; exp underflows to exactly 0
BIGB = 86.0  # row-invalid exp bias: elements stay finite, row sum -> inf

F32 = mybir.dt.float32
F32R = mybir.dt.float32r
AF = mybir.ActivationFunctionType
ALU = mybir.AluOpType

_NC = None


def _build():
    nc = bacc.Bacc("TRN2", target_bir_lowering=False, debug=False)

    xT = nc.dram_tensor("xT", [D, N], F32R, kind="ExternalInput").ap()
    w = nc.dram_tensor("w", [D, 3 * HL * DH], F32R, kind="ExternalInput").ap()
    bqk = nc.dram_tensor("bqk", [128, 8], F32, kind="ExternalInput").ap()
    bv = nc.dram_tensor("bv", [1, HL * DH], F32R, kind="ExternalInput").ap()
    wout = nc.dram_tensor("wout", [DH, HL, D], F32R, kind="ExternalInput").ap()
    amask = nc.dram_tensor("amask", [1, N], F32R, kind="ExternalInput").ap()
    onesd = nc.dram_tensor("onesd", [1, N], F32R, kind="ExternalInput").ap()
    bigb = nc.dram_tensor("bigb", [128, 8], F32, kind="ExternalInput").ap()
    cit = nc.dram_tensor("cit", [128, 8], F32, kind="ExternalInput").ap()
    identd = nc.dram_tensor("identd", [128, 128], F32R, kind="ExternalInput").ap()

    attn_o = nc.dram_tensor("attn", [HL, N, N], F32, kind="ExternalOutput").ap()
    outp_o = nc.dram_tensor("outp", [N, D], F32, kind="ExternalOutput").ap()

    with tile.TileContext(nc) as tc, ExitStack() as ctx:
        smalls = ctx.enter_context(tc.tile_pool(name="smalls", bufs=1))
        persist = ctx.enter_context(tc.tile_pool(name="persist", bufs=1))
        psum = ctx.enter_context(tc.tile_pool(name="psum", bufs=4, space="PSUM"))

        ident = smalls.tile([128, 128], F32R)
        nc.sync.dma_start(ident[:], identd)
        amask_sb = smalls.tile([1, N], F32R)
        nc.sync.dma_start(amask_sb[:], amask)
        ones_sb = smalls.tile([1, N], F32R)
        nc.sync.dma_start(ones_sb[:], onesd)
        bigb_sb = smalls.tile([128, 8], F32)
        nc.sync.dma_start(bigb_sb[:], bigb)
        cit_sb = smalls.tile([128, 8], F32)
        nc.sync.dma_start(cit_sb[:], cit)
        bqk_sb = smalls.tile([128, 8], F32)
        nc.sync.dma_start(bqk_sb[:], bqk)
        bv_sb = smalls.tile([1, HL * DH], F32R)
        nc.sync.dma_start(bv_sb[:], bv)

        qk_all = persist.tile([128, 8, N], F32R)  # q chunks 0..3, k chunks 4..7
        v_all = persist.tile([128, 8, HL * DH], F32R)  # token-major v
        outT_all = persist.tile([DH, HL, N], F32R)

        # ---------------- Phase 1: QKV projection ----------------
        with tc.tile_pool(name="ph1", bufs=1) as ph1:
            xt_all = ph1.tile([128, 8, N], F32R)
            w_all = ph1.tile([128, 8, 3 * HL * DH], F32R)
            nc.sync.dma_start(xt_all[:], xT.rearrange("(dc p) n -> p dc n", p=128))
            nc.sync.dma_start(w_all[:], w.rearrange("(dc p) f -> p dc f", p=128))

            for fc in range(8):  # q,k chunks: qk^T[f, tok]
                ps = psum.tile([128, N], F32, tag="big")
                for hn in (0, 512):
                    for d in range(8):
                        nc.tensor.matmul(
                            ps[:, hn : hn + 512],
                            w_all[:, d, fc * 128 : (fc + 1) * 128],
                            xt_all[:, d, hn : hn + 512],
                            start=(d == 0),
                            stop=(d == 7),
                        )
                nc.scalar.activation(
                    qk_all[:, fc, :], ps[:],
                    AF.Identity, bias=bqk_sb[:, fc : fc + 1], scale=1.0,
                )

            for t in range(8):  # v token-major (+ bias as rank-1 matmul)
                vps = psum.tile([128, 512], F32, tag="big")
                for d in range(8):
                    nc.tensor.matmul(
                        vps[:],
                        xt_all[:, d, t * 128 : (t + 1) * 128],
                        w_all[:, d, 2 * HL * DH : 3 * HL * DH],
                        start=(d == 0),
                        stop=False,
                    )
                nc.tensor.matmul(
                    vps[:], ones_sb[:, 0:128], bv_sb[:], start=False, stop=True
                )
                nc.vector.tensor_copy(v_all[:, t, :], vps[:])

        # ---------------- Phase 2: attention per head ----------------
        wpool = ctx.enter_context(tc.tile_pool(name="wpool", bufs=1))
        wout_sb = wpool.tile([DH, HL, D], F32R)
        nc.sync.dma_start(wout_sb[:], wout)

        ph2 = ExitStack()
        aug = ph2.enter_context(tc.tile_pool(name="aug", bufs=2))
        pexp = ph2.enter_context(tc.tile_pool(name="pexp", bufs=4))
        pattn = ph2.enter_context(tc.tile_pool(name="pattn", bufs=2))
        ptall = ph2.enter_context(tc.tile_pool(name="ptall", bufs=1))
        tiny = ph2.enter_context(tc.tile_pool(name="tiny", bufs=4))

        for l in range(HL):
            fcq, po = l // 2, (l % 2) * 64
            qaug = aug.tile([65, N], F32R, tag="qaug")
            kaug = aug.tile([65, N], F32R, tag="kaug")
            nc.sync.dma_start(qaug[0:64, :], qk_all[po : po + 64, fcq, :])
            nc.sync.dma_start(qaug[64:65, :], ones_sb[:])
            nc.sync.dma_start(kaug[0:64, :], qk_all[po : po + 64, 4 + fcq, :])
            nc.sync.dma_start(kaug[64:65, :], amask_sb[:])

            pt_all = ptall.tile([128, 8, N], F32R, tag="pt")

            for g in range(2):  # groups of 4 i-chunks share one reciprocal
                sums = tiny.tile([128, 4], F32, tag="sums")
                p_tiles = []
                for k in range(4):
                    ic = g * 4 + k
                    sps = psum.tile([128, N], F32, tag="big")
                    for hn in (0, 512):
                        nc.tensor.matmul(
                            sps[:, hn : hn + 512],
                            qaug[:, ic * 128 : (ic + 1) * 128],
                            kaug[:, hn : hn + 512],
                            start=True,
                            stop=True,
                        )
                    p_sb = pexp.tile([128, N], F32, tag="p")
                    nc.scalar.activation(
                        p_sb[:], sps[:], AF.Exp,
                        bias=bigb_sb[:, ic : ic + 1], scale=1.0,
                        accum_out=sums[:, k : k + 1],
                    )
                    p_tiles.append(p_sb)
                rec = tiny.tile([128, 4], F32, tag="rec")
                nc.vector.reciprocal(rec[:], sums[:])

                for k in range(4):
                    ic = g * 4 + k
                    attn_sb = pattn.tile([128, N], F32R, tag="attn")
                    nc.vector.tensor_scalar(
                        attn_sb[:], p_tiles[k][:],
                        rec[:, k : k + 1], cit_sb[:, ic : ic + 1],
                        op0=ALU.mult, op1=ALU.add,
                    )
                    nc.sync.dma_start(
                        attn_o[l, ic * 128 : (ic + 1) * 128, :],
                        attn_sb[:].bitcast(F32),
                    )
                    ptps = psum.tile([128, N], F32R, tag="big")
                    for jc in range(8):
                        nc.tensor.transpose(
                            ptps[:, jc * 128 : (jc + 1) * 128],
                            attn_sb[:, jc * 128 : (jc + 1) * 128],
                            ident[:],
                        )
                    dst = pt_all[:, :, ic * 128 : (ic + 1) * 128]
                    src = ptps[:].rearrange("p (g n) -> p g n", g=8)
                    if ic % 2 == 0:
                        nc.scalar.copy(dst, src)
                    else:
                        nc.vector.tensor_copy(dst, src)

            # P@V: outT[d, i] accumulated over j-chunks
            ovt = psum.tile([DH, N], F32, tag="big")
            for hn in (0, 512):
                for jc in range(8):
                    nc.tensor.matmul(
                        ovt[:, hn : hn + 512],
                        v_all[:, jc, l * DH : (l + 1) * DH],
                        pt_all[:, jc, hn : hn + 512],
                        start=(jc == 0),
                        stop=(jc == 7),
                    )
            nc.vector.tensor_copy(outT_all[:, l, :], ovt[:])

        ph2.close()

        # ---------------- Phase 3: output projection (partial) ----------------
        with tc.tile_pool(name="pout", bufs=2) as pout:
            for ic in range(8):
                ops = psum.tile([128, D], F32, tag="big")
                for hn in (0, 512):
                    for l in range(HL):
                        nc.tensor.matmul(
                            ops[:, hn : hn + 512],
                            outT_all[:, l, ic * 128 : (ic + 1) * 128],
                            wout_sb[:, l, hn : hn + 512],
                            start=(l == 0),
                            stop=(l == HL - 1),
                        )
                osb = pout.tile([128, D], F32, tag="o")
                nc.vector.tensor_copy(osb[:], ops[:])
                nc.sync.dma_start(outp_o[ic * 128 : (ic + 1) * 128, :], osb[:])

    nc.compile()
    return nc


def _get_nc():
    global _NC
    if _NC is None:
        _NC = _build()
    return _NC


def _prep_core(x, mask, Wqkv, bqkv, Wout, c):
    b, half = c // 2, c % 2
    s, e = half * 512, (half + 1) * 512

    xT = np.ascontiguousarray(x[b].T)

    Wq = Wqkv[:, s:e] * np.float32(SCALE)
    Wk = Wqkv[:, D + s : D + e]
    Wv = Wqkv[:, 2 * D + s : 2 * D + e]
    w = np.ascontiguousarray(np.concatenate([Wq, Wk, Wv], axis=1))

    bq = bqkv[s:e] * np.float32(SCALE)
    bk = bqkv[D + s : D + e]
    bqk = np.ascontiguousarray(np.concatenate([bq, bk]).reshape(8, 128).T)
    bv = np.ascontiguousarray(bqkv[2 * D + s : 2 * D + e].reshape(1, 512))

    wout = np.ascontiguousarray(Wout[s:e, :].reshape(HL, DH, D).transpose(1, 0, 2))

    m = np.concatenate([[True], mask[b]])  # [N]
    amask = np.where(m, 0.0, NEG).astype(np.float32).reshape(1, N)
    rv = m.astype(np.float32)
    bigb = np.ascontiguousarray(((1.0 - rv) * BIGB).astype(np.float32).reshape(8, 128).T)
    cit = np.ascontiguousarray(((1.0 - rv) / N).astype(np.float32).reshape(8, 128).T)

    return {
        "xT": xT.astype(np.float32),
        "w": w.astype(np.float32),
        "bqk": bqk.astype(np.float32),
        "bv": bv.astype(np.float32),
        "wout": wout.astype(np.float32),
        "amask": amask,
        "onesd": np.ones((1, N), np.float32),
        "bigb": bigb,
        "cit": cit,
        "identd": np.eye(128, dtype=np.float32),
    }


def kernel(x, mask, Wqkv, bqkv, Wout, bout):
    x = np.asarray(x, dtype=np.float32)
    mask = np.asarray(mask)
    Wqkv = np.asarray(Wqkv, dtype=np.float32)
    bqkv = np.asarray(bqkv, dtype=np.float32)
    Wout = np.asarray(Wout, dtype=np.float32)
    bout = np.asarray(bout, dtype=np.float32)

    nc = _get_nc()
    in_maps = [_prep_core(x, mask, Wqkv, bqkv, Wout, c) for c in range(8)]
    res = run_bass_kernel_spmd(nc, in_maps, core_ids=list(range(8)))

    attn = np.empty((B, H, N, N), np.float32)
    out = np.empty((B, N, D), np.float32)
    for c in range(8):
        b, half = c // 2, c % 2
        attn[b, half * HL : (half + 1) * HL] = res.results[c]["attn"]
    for b in range(B):
        out[b] = res.results[2 * b]["outp"] + res.results[2 * b + 1]["outp"] + bout
    return (out, attn)
